# revision 27
# baseline (speedup 1.0000x reference)
"""Multi-head causal attention block (B=2, S=2048, F=1024, H=16, D=64)
on 8 TRN2 NeuronCores.

Sharding: core = 4*b + g  (b = batch 0..1, g = head-group 0..3, 4 heads each).
Each core computes, for its batch and its 4 heads:
  qkv projection (columns of w_attn for its heads), causal attention,
  and the partial output projection (rows of w_proj for its heads).
Host sums the 4 per-group partials per batch and adds the bias constant
(b_proj + b_attn_v @ w_proj, which is token-independent).

v3:
  * all matmul operands bf16 (FWL weight loads overlap with matmuls; the
    f32r baseline paid a serial ~140us LDWEIGHTS tax), half the HBM
    traffic for inputs.
  * x is transposed on the HOST; xT [f, s] loads as plain contiguous
    DMAs (no PE transposes, no XBAR transposes, no transpose evacs).
  * input DMA issue split across the sync and vector queues so the
    first qkproj can start ~6us in.
  * engine rebalance: ACT does exp + the qkproj evacuation (fused
    per-partition bias add); DVE does vproj/outproj evacuations,
    reciprocal, the zps->SBUF evac and the normalize multiply;
    GPSIMD does the causal diag masks (affine_select zeroing the exp
    output triangle) and the denominator partition broadcasts.
  * normalize releases the zps PSUM bank early (recip + copy out, then
    broadcast/multiply from SBUF) so the next head-pair's AV matmuls
    are not stalled behind the full normalize chain.
  * exp LUT preloaded by a dummy activation at t=0.

On-chip dataflow (orientation B -- scores transposed):
  xT   [f, s]   host-pretransposed, plain DMA
  qkT  [dim, s] = wqk^T @ xT; chunks [q_h0|q_h1],[q_h2|q_h3],[k_h0|k_h1],[k_h2|k_h3]
  v    [s, d]   direct orientation, +ones column per head (denominator row)
  attention per head h, sq-chunk c (512 wide), sk tile t<=diag:
    sT = matmul(lhsT=kT_h[:,t], rhs=qT_h[:,chunk])  [sk=128, sq<=512] PSUM
    (pairs of t share one 2-bank PSUM tile; one exp op per pair)
    exp on ACT -> SBUF bf16; diag triangle zeroed by GPSIMD affine_select
    zT'[65, 512] += v_ones_h[:,t].T @ expP  (PSUM accumulate; row 64 = denom)
    normalize: z = zT'[:64] * bcast(approx_recip(zT'[64]))
  out partial [s, f] = zTm.T @ wp
"""

import numpy as np
import ml_dtypes

import concourse.mybir as mybir
import concourse.tile as tile
from concourse import bacc
from concourse.bass_utils import run_bass_kernel_spmd

B, S, F, H, D = 2, 2048, 1024, 16, 64
P = 128
NCORES = 8
HPC = 4  # heads per core
GD = HPC * D  # 256 dims per head group
ST = S // P  # 16 sequence tiles
FC = F // P  # 8 feature chunks
SQC = 4  # sq chunks of 512
CW = 512  # chunk width
NEG = -1.0e9

f32 = mybir.dt.float32
bf16 = mybir.dt.bfloat16

# fallbacks for HW quirks (set from micro-diagnostics: ACT Identity with a
# bias AP and reciprocal-from-PSUM both give wrong results on real HW)
MASKS = "gpsimd"   # "gpsimd" affine_select post-exp | "pe" mask matmul pre-exp
BIAS = "dve"       # "act" Identity+bias AP evac | "dve" tensor_tensor add
RECIP_SRC = "sbuf"  # "psum" recip direct from PSUM | "sbuf" via SBUF copy

_cached_nc = None


def build_nc():
    nc = bacc.Bacc("TRN2", target_bir_lowering=False, debug=False,
                   num_devices=NCORES)
    xt_d = nc.dram_tensor("xt", [F, S], bf16, kind="ExternalInput")
    wqk = nc.dram_tensor("wqk", [F, 2 * GD], bf16, kind="ExternalInput")
    wv = nc.dram_tensor("wv", [F, GD], bf16, kind="ExternalInput")
    wp = nc.dram_tensor("wp", [GD, F], bf16, kind="ExternalInput")
    bqk = nc.dram_tensor("bqk", [P, 4], f32, kind="ExternalInput")
    out = nc.dram_tensor("out", [S, F], f32, kind="ExternalOutput")

    with tile.TileContext(nc) as tc:
        with (
            tc.tile_pool(name="consts", bufs=1) as consts,
            tc.tile_pool(name="stage", bufs=1) as stage,
            tc.tile_pool(name="work", bufs=3) as work,
            tc.tile_pool(name="eps", bufs=6) as eps,
            tc.tile_pool(name="norm", bufs=6) as norm,
            tc.tile_pool(name="normr", bufs=2) as normr,
            tc.tile_pool(name="ps_s", bufs=2, space="PSUM") as ps_s,
            tc.tile_pool(name="ps_z", bufs=2, space="PSUM") as ps_z,
            tc.tile_pool(name="ps_m", bufs=2, space="PSUM") as ps_m,
        ):
            bqk_sb = consts.tile([P, 4], f32)
            nc.sync.dma_start(bqk_sb[:], bqk[:])
            ones64 = consts.tile([1, D], bf16)
            nc.gpsimd.memset(ones64[:], 1.0)

            if MASKS == "pe":
                from concourse.masks import make_identity
                ident_f = consts.tile([P, P], f32)
                make_identity(nc, ident_f[:])
                ident = consts.tile([P, P], bf16)
                nc.vector.tensor_copy(ident[:], ident_f[:])
                negl_f = consts.tile([P, P], f32)
                nc.gpsimd.memset(negl_f[:], 0.0)
                nc.gpsimd.affine_select(
                    out=negl_f[:], in_=negl_f[:],
                    compare_op=mybir.AluOpType.is_ge,
                    fill=NEG, base=0,
                    pattern=[[1, P]], channel_multiplier=-1,
                )
                negl = consts.tile([P, P], bf16)
                nc.vector.tensor_copy(negl[:], negl_f[:])

            # ---- persistent tiles ----
            xT = stage.tile([P, FC, S], bf16, tag="xT", name="xT")
            qkT = stage.tile([P, 4, S], bf16, tag="qkT", name="qkT")
            vt = stage.tile([P, HPC, ST, D + 1], bf16, tag="vt", name="vt")
            zTm = stage.tile([P, 2, S], bf16, tag="zTm", name="zTm")
            wqk_sb = stage.tile([P, FC, 2 * GD], bf16, tag="wqk", name="wqk_sb")
            wv_sb = stage.tile([P, FC, GD], bf16, tag="wv", name="wv_sb")
            wp_sb = stage.tile([P, 2, F], bf16, tag="wp", name="wp_sb")

            for h in range(HPC):
                nc.gpsimd.memset(vt[:, h, :, D:D + 1], 1.0)

            # ---- input DMAs: one consolidated DMA per chunk / weight
            # tensor ([p][fc][cols] gather patterns), chunk-0 + wqk first ----
            def xt_dma(eng, c):
                eng.dma_start(
                    xT[:, :, c * CW:(c + 1) * CW],
                    xt_d.rearrange("(fc p) s -> p fc s", p=P)[
                        :, :, c * CW:(c + 1) * CW],
                )

            xt_dma(nc.sync, 0)
            nc.scalar.dma_start(
                wqk_sb[:], wqk.rearrange("(fc p) c -> p fc c", p=P))
            nc.scalar.dma_start(
                wv_sb[:], wv.rearrange("(fc p) c -> p fc c", p=P))
            for c in range(1, SQC):
                xt_dma(nc.sync, c)
            nc.scalar.dma_start(
                wp_sb[:], wp.rearrange("(cc p) f -> p cc f", p=P))

            # preload the exp LUT (after the weight DMA issues so the
            # ~2.7us table load does not delay them)
            warm = consts.tile([1, 8], f32)
            nc.gpsimd.memset(warm[:], 0.0)
            nc.scalar.activation(warm[:], warm[:],
                                 mybir.ActivationFunctionType.Exp)

            # ---- per-chunk projection tasks ----
            def qkproj_task(c, oc):
                pp = ps_m.tile([P, CW], f32, tag="mps", name="qkps")
                for fc in range(FC):
                    nc.tensor.matmul(
                        pp[:],
                        wqk_sb[:, fc, oc * P:(oc + 1) * P],
                        xT[:, fc, c * CW:(c + 1) * CW],
                        start=(fc == 0), stop=(fc == FC - 1),
                    )
                if BIAS == "act":
                    # fused PSUM->SBUF evac + per-partition bias + bf16 cast
                    nc.scalar.activation(
                        qkT[:, oc, c * CW:(c + 1) * CW], pp[:],
                        mybir.ActivationFunctionType.Identity,
                        bias=bqk_sb[:, oc:oc + 1], scale=1.0,
                    )
                else:
                    nc.vector.tensor_tensor(
                        qkT[:, oc, c * CW:(c + 1) * CW], pp[:],
                        bqk_sb[:, oc:oc + 1].to_broadcast((P, CW)),
                        mybir.AluOpType.add,
                    )

            def vproj_task(c, tt):
                t = 4 * c + tt
                pp = ps_m.tile([P, GD], f32, tag="mps", name="vps")
                for fc in range(FC):
                    nc.tensor.matmul(
                        pp[:],
                        xT[:, fc, t * P:(t + 1) * P],
                        wv_sb[:, fc, :],
                        start=(fc == 0), stop=(fc == FC - 1),
                    )
                nc.vector.tensor_copy(
                    vt[:, :, t, :D],
                    pp[:].rearrange("p (h d) -> p h d", h=HPC),
                )

            def av(zp, h, t, ep_ap, col0, ncols, start, stop):
                nc.tensor.matmul(
                    zp[:D + 1, col0:col0 + ncols],
                    vt[:, h, t, :],
                    ep_ap,
                    start=start, stop=stop,
                    skip_group_check=True,
                )

            def scores(sp_ap, h, t, c, q0, qw, stop=True):
                lo = (h % 2) * D
                nc.tensor.matmul(
                    sp_ap,
                    qkT[lo:lo + D, 2 + h // 2, t * P:(t + 1) * P],
                    qkT[lo:lo + D, h // 2, c * CW + q0:c * CW + q0 + qw],
                    start=True, stop=stop,
                    skip_group_check=True,
                )

            def pe_mask(sp_ap):
                nc.tensor.matmul(
                    sp_ap, ident[:], negl[:],
                    start=False, stop=True,
                    skip_group_check=True,
                )

            def diag_zero(ep_ap):
                # zero the exp-output block where col j < partition p
                nc.gpsimd.affine_select(
                    out=ep_ap, in_=ep_ap,
                    compare_op=mybir.AluOpType.is_ge,
                    fill=0.0, base=0,
                    pattern=[[1, P]], channel_multiplier=-1,
                )

            def attention(c, fillers):
                # insertion points: one after each pair's exp / AV emission
                npts = 2 * (2 * c + 2) * 2
                state = {"fi": 0, "pt": 0}
                posts = []

                def fill():
                    state["pt"] += 1
                    left = npts - state["pt"] + 1
                    remaining = len(fillers) - state["fi"]
                    k = (remaining + left - 1) // left if left > 0 else remaining
                    for _ in range(k):
                        fillers[state["fi"]]()
                        state["fi"] += 1

                for hp in range(2):
                    heads = (2 * hp, 2 * hp + 1)
                    zps = [
                        ps_z.tile([P, CW], f32, tag="zps", name=f"zps{i}")
                        for i in range(2)
                    ]
                    # off-diagonal pairs (full width); both heads' score
                    # matmuls issued adjacently so the K=64 matmuls pack
                    # into disjoint PE row groups and run concurrently.
                    for pair in range(2 * c):
                        t0, t1 = 2 * pair, 2 * pair + 1
                        sp2 = [
                            ps_s.tile([P, 2 * CW], f32, tag="sps",
                                      name=f"sps{i}")
                            for i in range(2)
                        ]
                        # interleave heads per tile: adjacent matmuls hit
                        # disjoint PE row groups and co-execute
                        for i, h in enumerate(heads):
                            scores(sp2[i][:, 0:CW], h, t0, c, 0, CW)
                        for i, h in enumerate(heads):
                            scores(sp2[i][:, CW:2 * CW], h, t1, c, 0, CW)
                        ep2 = []
                        for i, h in enumerate(heads):
                            ep = eps.tile([P, 2 * CW], bf16, tag="ep",
                                          name=f"ep{i}")
                            nc.scalar.activation(
                                ep[:], sp2[i][:],
                                mybir.ActivationFunctionType.Exp,
                            )
                            ep2.append(ep)
                        fill()
                        first = (t0 == 0)
                        for i, h in enumerate(heads):
                            av(zps[i], h, t0, ep2[i][:, 0:CW], 0, CW,
                               first, False)
                            av(zps[i], h, t1, ep2[i][:, CW:2 * CW], 0, CW,
                               False, False)
                        fill()
                    # diagonal pairs: widths (512, 384) and (256, 128)
                    for dp in range(2):
                        ta, tb = 4 * c + 2 * dp, 4 * c + 2 * dp + 1
                        offa, offb = 2 * dp * P, (2 * dp + 1) * P
                        wa, wb = CW - offa, CW - offb
                        sp2 = [
                            ps_s.tile([P, 2 * CW], f32, tag="sps",
                                      name=f"sps{i}")
                            for i in range(2)
                        ]
                        if MASKS == "pe":
                            for i, h in enumerate(heads):
                                scores(sp2[i][:, 0:wa], h, ta, c, offa, wa,
                                       stop=False)
                                pe_mask(sp2[i][:, 0:P])
                                scores(sp2[i][:, wa:wa + wb], h, tb, c, offb,
                                       wb, stop=False)
                                pe_mask(sp2[i][:, wa:wa + P])
                        else:
                            for i, h in enumerate(heads):
                                scores(sp2[i][:, 0:wa], h, ta, c, offa, wa)
                            for i, h in enumerate(heads):
                                scores(sp2[i][:, wa:wa + wb], h, tb, c,
                                       offb, wb)
                        ep2 = []
                        for i, h in enumerate(heads):
                            ep = eps.tile([P, 2 * CW], bf16, tag="ep",
                                          name=f"ep{i}")
                            nc.scalar.activation(
                                ep[:, 0:wa + wb], sp2[i][:, 0:wa + wb],
                                mybir.ActivationFunctionType.Exp,
                            )
                            if MASKS == "gpsimd":
                                diag_zero(ep[:, 0:P])
                                diag_zero(ep[:, wa:wa + P])
                            ep2.append(ep)
                        fill()
                        first = (c == 0 and dp == 0)
                        for i, h in enumerate(heads):
                            av(zps[i], h, ta, ep2[i][:, 0:wa], offa, wa,
                               first, False)
                            av(zps[i], h, tb, ep2[i][:, wa:wa + wb], offb,
                               wb, False, (dp == 1))
                        fill()
                    # normalize: free each zps bank ASAP with ONE copy to
                    # SBUF (row 0 = denominator); the reciprocal, broadcast
                    # (a PE rank-1 outer product -- keeps GPSIMD single-
                    # ucode so its affine_select library never swaps out)
                    # and multiply are deferred into the next chunk's
                    # filler stream (zTm is only read 1-2 chunks later)
                    for i, h in enumerate(heads):
                        den = norm.tile([1, CW], f32, tag="den", name="den")
                        nc.vector.tensor_copy(den[:], zps[i][D:D + 1, :])
                        zsb = norm.tile([D, CW], f32, tag="zsb", name="zsb")
                        nc.vector.tensor_copy(zsb[:], zps[i][:D])

                        def normpost(h=h, den=den, zsb=zsb, c=c):
                            rec = normr.tile([1, CW], f32, tag="rec",
                                             name="rec")
                            nc.vector.reciprocal_approx_fast(
                                rec[:], den[:])
                            recbf = normr.tile([1, CW], bf16, tag="recbf",
                                               name="recbf")
                            nc.vector.tensor_copy(recbf[:], rec[:])
                            rpp = ps_m.tile([D, CW], f32, tag="mps",
                                            name="rps")
                            nc.tensor.matmul(
                                rpp[:], ones64[:], recbf[:],
                                start=True, stop=True,
                            )
                            lo = (h % 2) * D
                            nc.vector.tensor_mul(
                                zTm[lo:lo + D, h // 2, c * CW:(c + 1) * CW],
                                zsb[:], rpp[:],
                            )
                        posts.append(normpost)
                while state["fi"] < len(fillers):
                    fillers[state["fi"]]()
                    state["fi"] += 1
                return posts

            def outproj_task(c, tt, n):
                t = 4 * c + tt
                osb = work.tile([P, CW], f32, tag="osb", name="osb")
                pp = ps_m.tile([P, CW], f32, tag="mps", name="ops")
                for cc in range(2):
                    nc.tensor.matmul(
                        pp[:],
                        zTm[:, cc, t * P:(t + 1) * P],
                        wp_sb[:, cc, n * CW:(n + 1) * CW],
                        start=(cc == 0), stop=(cc == 1),
                    )
                # split evac between DVE and ACT, out-DMA issue between
                # the sync and scalar queues (shorter serial tail)
                if n == 0:
                    nc.vector.tensor_copy(osb[:], pp[:])
                    nc.sync.dma_start(
                        out[t * P:(t + 1) * P, n * CW:(n + 1) * CW], osb[:]
                    )
                else:
                    nc.scalar.copy(osb[:], pp[:])
                    nc.scalar.dma_start(
                        out[t * P:(t + 1) * P, n * CW:(n + 1) * CW], osb[:]
                    )

            def prep_tasks(c):
                tasks = [(lambda oc=oc: qkproj_task(c, oc)) for oc in range(4)]
                tasks += [(lambda tt=tt: vproj_task(c, tt)) for tt in range(4)]
                return tasks

            # chunk 0 prep up front, then software-pipeline: during
            # attention(c), weave in outproj(c-1) and all prep for c+1.
            for task in prep_tasks(0):
                task()
            # outproj(c) is shifted as late as possible so the long final
            # attention chunks (most insertion points) have filler work:
            # att0: prep1, att1: prep2, att2: prep3+out0, att3: out1+out2
            out_t = [
                [(lambda tt=tt, cp=cp, n=n: outproj_task(cp, tt, n))
                 for tt in range(4) for n in range(2)]
                for cp in range(SQC)
            ]
            posts = []
            for c in range(SQC):
                fillers = list(posts)  # previous chunk's recip/bcast/mul
                if c + 1 < SQC:
                    fillers += prep_tasks(c + 1)
                if c == 2:
                    fillers += out_t[0]
                if c == 3:
                    fillers += out_t[1] + out_t[2]
                posts = attention(c, fillers)
            for p_ in posts:
                p_()
            for task in out_t[3]:
                task()
    nc.compile()
    return nc


def make_in_maps(x, w_attn, b_attn, w_proj):
    bf = ml_dtypes.bfloat16
    x = np.ascontiguousarray(np.asarray(x, dtype=np.float32))
    w_attn = np.asarray(w_attn, dtype=np.float32)
    b_attn = np.asarray(b_attn, dtype=np.float32)
    w_proj = np.ascontiguousarray(np.asarray(w_proj, dtype=np.float32))
    scale = np.float32(1.0 / np.sqrt(D))
    xtb = [np.ascontiguousarray(x[b].T).astype(bf) for b in range(B)]
    in_maps = []
    for core in range(NCORES):
        b, g = divmod(core, 4)
        sl = slice(g * GD, (g + 1) * GD)
        wq = w_attn[:, sl] * scale
        wk = w_attn[:, F + g * GD:F + (g + 1) * GD]
        wqkm = np.ascontiguousarray(
            np.concatenate([wq, wk], axis=1)
        ).astype(bf)
        wvm = np.ascontiguousarray(
            w_attn[:, 2 * F + g * GD:2 * F + (g + 1) * GD]
        ).astype(bf)
        wpg = np.ascontiguousarray(w_proj[sl, :]).astype(bf)
        bq = b_attn[sl] * scale
        bk = b_attn[F + g * GD:F + (g + 1) * GD]
        bqkm = np.ascontiguousarray(
            np.concatenate([bq, bk]).reshape(4, P).T, dtype=np.float32
        )
        in_maps.append(
            {"xt": xtb[b], "wqk": wqkm, "wv": wvm, "wp": wpg, "bqk": bqkm}
        )
    return in_maps


def assemble(results, b_attn, b_proj, w_proj):
    b_attn = np.asarray(b_attn, dtype=np.float64)
    b_proj = np.asarray(b_proj, dtype=np.float64)
    w_proj = np.asarray(w_proj, dtype=np.float64)
    const = b_attn[2 * F:] @ w_proj + b_proj  # token-independent v-bias term
    full = np.empty((B, S, F), dtype=np.float32)
    for b in range(B):
        acc = results[4 * b]["out"].astype(np.float64)
        for g in range(1, 4):
            acc = acc + results[4 * b + g]["out"]
        full[b] = (acc + const).astype(np.float32)
    return full


def kernel(x, w_attn, b_attn, w_proj, b_proj):
    global _cached_nc
    if _cached_nc is None:
        _cached_nc = build_nc()
    in_maps = make_in_maps(x, w_attn, b_attn, w_proj)
    res = run_bass_kernel_spmd(
        _cached_nc, in_maps, core_ids=list(range(NCORES))
    )
    return assemble(res.results, b_attn, b_proj, w_proj)
